# revision 26
# baseline (speedup 1.0000x reference)
"""GatedTSNorm Trainium2 kernel (v3, bf16 datapath + blocked parallel scans).

Math: the gated EMA y[t] = (1-g~[t])*y[t-1] + g~[t]*v[t] is linear with a
gate shared across channels, so channel-weighted sums commute with it:
    mean[b,t] = EMA(sum_c wa_c x[b,c,t])
    var[b,t]  = EMA(sb - mean*(2*ub - mean))
with ua/ub/sb the wa/wb-weighted reductions of x and x^2 over C.
Output: out[c,t] = Wo_w[c] * alpha[t] * (x[c,t] - mean[t]) + Wo_b[c],
alpha = rsqrt(var + eps).

Sharding: batch B=8 -> one batch per NeuronCore, zero communication.

Per-core pipeline (c-major, C=512 on 4 partition tiles, bf16 datapath,
TC=2048 time chunks = 4 MiB loads / 2 MiB bf16 stores):
  SWDGE cast-DMA x f32->bf16 -> ScalarE x^2 (per 512 subchunk) -> TensorE
  reductions (bf16 stationary [wa|wb|0] / [0|0|wb]) -> rows evac'd f32.
  Per chunk the rows are DMA-packed to (128, 16) and both EMAs run as
  blocked parallel scans: 128 time-blocks scan in parallel on VectorE, the
  128 block carries are fixed up with one (1,128) scan whose inputs come
  via a TensorE transpose (PSUM operand) and whose output lands directly
  in the carry row; y = cumA * carry + local is one scalar_tensor_tensor
  with the carry column (PSUM) as per-partition scalar.  mean/alpha are
  unpacked (cast bf16) to rows, partition-broadcast on GpSimd, and the
  output is two VectorE bf16 passes (y = x - mean_bc; z = y * alpha_bc)
  + a ScalarE per-channel affine (out = Wo_w*z + Wo_b), stored as bf16
  (upcast to f32 on host; rel tolerance is 2e-2).
"""

import numpy as np

MOMENTUM = 0.05
EPS = 1e-06

B, C, T = 8, 512, 8192
NCT = C // 128          # 4 partition tiles of channels
TC = 2048               # time chunk == row chunk
SUB = 512               # matmul / psum subchunk
NSC = TC // SUB
L = TC // 128           # packed elems per partition per chunk

_PROG_CACHE = {}


def _build_program(t_total, reps=1, mode="full"):
    import concourse.bacc as bacc
    import concourse.bass as bass
    import concourse.tile as tile
    from concourse import mybir
    from contextlib import ExitStack

    f32 = mybir.dt.float32
    bf16 = mybir.dt.bfloat16
    AF = mybir.ActivationFunctionType
    OP = mybir.AluOpType

    nch = t_total // TC

    nc = bacc.Bacc(None, target_bir_lowering=False)

    x_d = nc.dram_tensor("x", [C, t_total], f32, kind="ExternalInput")
    hr_d = nc.dram_tensor("hrows", [128, 3, nch * L], f32,
                          kind="ExternalInput")
    ar_d = nc.dram_tensor("arows", [1, nch * 128], f32, kind="ExternalInput")
    w3a_d = nc.dram_tensor("w3a", [128, NCT, 3], bf16, kind="ExternalInput")
    w3b_d = nc.dram_tensor("w3b", [128, NCT, 3], bf16, kind="ExternalInput")
    wsc_d = nc.dram_tensor("wscale", [128, NCT], f32, kind="ExternalInput")
    wbi_d = nc.dram_tensor("wbias", [128, NCT], f32, kind="ExternalInput")
    out_d = nc.dram_tensor("out", [C, t_total], bf16, kind="ExternalOutput")

    x_v = x_d.rearrange("(ct p) t -> p ct t", p=128)
    out_v = out_d.rearrange("(ct p) t -> p ct t", p=128)

    with tile.TileContext(nc) as tc, ExitStack() as ctx, \
            nc.allow_low_precision(reason="bf16 datapath; output metric is "
                                   "max-abs rel err vs global max, tol 2e-2"):
        const = ctx.enter_context(tc.tile_pool(name="const", bufs=1))
        xp = ctx.enter_context(tc.tile_pool(name="xp", bufs=4))
        xsqp = ctx.enter_context(tc.tile_pool(name="xsqp", bufs=3))
        yp = ctx.enter_context(tc.tile_pool(name="yp", bufs=2))
        zo = ctx.enter_context(tc.tile_pool(name="zo", bufs=2))
        rowsp = ctx.enter_context(tc.tile_pool(name="rowsp", bufs=2))
        rpk = ctx.enter_context(tc.tile_pool(name="rpk", bufs=2))
        dk = ctx.enter_context(tc.tile_pool(name="dk", bufs=2))
        ych = ctx.enter_context(tc.tile_pool(name="ych", bufs=2))
        rowbf = ctx.enter_context(tc.tile_pool(name="rowbf", bufs=2))
        bcp = ctx.enter_context(tc.tile_pool(name="bcp", bufs=2))
        ps_red = ctx.enter_context(
            tc.tile_pool(name="ps_red", bufs=2, space="PSUM"))
        ps_t = ctx.enter_context(
            tc.tile_pool(name="ps_t", bufs=1, space="PSUM"))
        ps_bc = ctx.enter_context(
            tc.tile_pool(name="ps_bc", bufs=3, space="PSUM"))

        # ---- constants ----
        ones_t = const.tile([128, 128], f32)
        nc.vector.memset(ones_t, 1.0)
        ident = const.tile([128, 128], f32)
        nc.gpsimd.affine_select(
            out=ident, in_=ones_t, pattern=[[1, 128]],
            compare_op=OP.is_equal, fill=0.0, base=0, channel_multiplier=-1,
        )
        one2 = const.tile([1, 2], f32)
        nc.vector.memset(one2, 1.0)
        ones_col = const.tile([1, 128], bf16)
        nc.vector.tensor_copy(out=ones_col, in_=ones_t[0:1, :])
        eps_t = const.tile([128, 1], f32)
        nc.vector.memset(eps_t, EPS)

        w3a = const.tile([128, NCT, 3], bf16)
        nc.sync.dma_start(out=w3a, in_=w3a_d[:])
        w3b = const.tile([128, NCT, 3], bf16)
        nc.sync.dma_start(out=w3b, in_=w3b_d[:])
        wsc = const.tile([128, NCT], f32)
        nc.sync.dma_start(out=wsc, in_=wsc_d[:])
        wbi = const.tile([128, NCT], f32)
        nc.sync.dma_start(out=wbi, in_=wbi_d[:])
        hr = const.tile([128, 3, nch * L], f32)
        nc.sync.dma_start(out=hr, in_=hr_d[:])
        arows = const.tile([1, nch * 128], f32)
        nc.sync.dma_start(out=arows, in_=ar_d[:])

        rep_cm = (tc.For_i(0, reps, 1, staggered_reset=True)
                  if reps > 1 else None)
        if rep_cm is not None:
            rep_cm.__enter__()

        st_rows = {}
        st_x = {}
        st_row_bf = {}

        def chunk_a(k):
            """load + square + channel reductions for chunk k."""
            t0 = k * TC
            x_ch = xp.tile([128, NCT, TC], bf16, name="x_ch")
            nc.gpsimd.dma_start(out=x_ch, in_=x_v[:, :, t0:t0 + TC])
            rows = rowsp.tile([3, TC], f32, tag="rows", name="rows")
            st_rows[k] = rows
            for sx in range(NSC):
                sl = slice(sx * SUB, (sx + 1) * SUB)
                xs = xsqp.tile([128, NCT, SUB], bf16, name="xs")
                nc.scalar.activation(out=xs, in_=x_ch[:, :, sl],
                                     func=AF.Square)
                rp = ps_red.tile([3, SUB], f32, name="red_ps")
                for ct in range(NCT):
                    nc.tensor.matmul(rp, w3a[:, ct, :], x_ch[:, ct, sl],
                                     start=(ct == 0), stop=False)
                for ct in range(NCT):
                    nc.tensor.matmul(rp, w3b[:, ct, :], xs[:, ct, :],
                                     start=False, stop=(ct == NCT - 1))
                nc.scalar.activation(out=rows[:, sl], in_=rp, func=AF.Copy)
            st_x[k] = x_ch

        def blocked_scan(a_ap, cA_ap, arow_ap, b_ap, carry_prev, tag):
            """EMA y[t] = a[t]*y[t-1] + b[t] over one packed chunk.

            128 blocks of L scan in parallel; block carries fixed up with a
            (1,128) scan via a TensorE transpose. carry_prev is a (1,1) AP
            holding the global carry entering this chunk (or None).
            Returns (y, carry_out AP)."""
            loc = dk.tile([128, L], f32, tag=f"loc{tag}", name=f"loc{tag}")
            nc.vector.tensor_tensor_scan(
                out=loc, data0=a_ap, data1=b_ap, initial=0.0,
                op0=OP.mult, op1=OP.add)
            psL = ps_t.tile([1, 128], f32, tag="pst", name="psL")
            nc.tensor.matmul(psL, loc[:, L - 1:L], ident,
                             start=True, stop=True)
            # cbuf[0] = carry_in; cbuf[1:129] = Y[0..127] (block-end scan)
            cbuf = ych.tile([1, 129], f32, tag=f"cbuf{tag}",
                            name=f"cbuf{tag}")
            if carry_prev is None:
                nc.vector.memset(cbuf[:, 0:1], 0.0)
            else:
                nc.vector.tensor_copy(out=cbuf[:, 0:1], in_=carry_prev)
            nc.vector.tensor_tensor_scan(
                out=cbuf[:, 1:129], data0=arow_ap, data1=psL,
                initial=(0.0 if carry_prev is None else carry_prev),
                op0=OP.mult, op1=OP.add)
            psc = ps_t.tile([128, 2], f32, tag="psc", name="psc")
            nc.tensor.matmul(psc, cbuf[:, 0:128], one2,
                             start=True, stop=True)
            y = dk.tile([128, L], f32, tag=f"y{tag}", name=f"y{tag}")
            nc.vector.scalar_tensor_tensor(
                out=y, in0=cA_ap, scalar=psc[:, 0:1], in1=loc,
                op0=OP.mult, op1=OP.add)
            return y, cbuf[:, 128:129]

        def dance(k, mcar, vcar):
            """chunk k: pack reductions, both EMA scans, mean/alpha rows."""
            if mode == "nodance":
                m_bf = rowbf.tile([1, TC], bf16, tag="mrow", name="m_bf")
                nc.vector.memset(m_bf, 0.0)
                a_bf = rowbf.tile([1, TC], bf16, tag="arow", name="a_bf")
                nc.vector.memset(a_bf, 1.0)
                st_row_bf[k] = (m_bf, a_bf)
                return mcar, vcar
            sl = slice(k * L, (k + 1) * L)
            gt = hr[:, 0, sl]
            a_ap = hr[:, 1, sl]
            cA = hr[:, 2, sl]
            arow_ap = arows[:, k * 128:(k + 1) * 128]
            pkt = rpk.tile([128, 3, L], f32, name="pkt")
            for j in range(3):
                nc.sync.dma_start(out=pkt[:, j, :],
                                  in_=st_rows.pop(k)[j:j + 1, :]
                                  if j == 2 else st_rows[k][j:j + 1, :])
            b1 = dk.tile([128, L], f32, tag="b1")
            nc.vector.tensor_tensor(out=b1, in0=gt, in1=pkt[:, 0, :],
                                    op=OP.mult)
            mean, mco = blocked_scan(a_ap, cA, arow_ap, b1, mcar, "m")
            # mean row can unpack + broadcast while the var scan runs
            m_pk = dk.tile([128, L], bf16, tag="mpk")
            nc.vector.tensor_copy(out=m_pk, in_=mean)
            m_bf = rowbf.tile([1, TC], bf16, tag="mrow", name="m_bf")
            nc.sync.dma_start(out=m_bf, in_=m_pk)
            t1 = dk.tile([128, L], f32, tag="t1")
            nc.vector.scalar_tensor_tensor(
                out=t1, in0=pkt[:, 1, :], scalar=2.0, in1=mean,
                op0=OP.mult, op1=OP.subtract)
            t2 = dk.tile([128, L], f32, tag="t2")
            nc.vector.tensor_tensor(out=t2, in0=t1, in1=mean, op=OP.mult)
            v = dk.tile([128, L], f32, tag="v")
            nc.vector.tensor_tensor(out=v, in0=pkt[:, 2, :], in1=t2,
                                    op=OP.subtract)
            b2 = dk.tile([128, L], f32, tag="b2")
            nc.vector.tensor_tensor(out=b2, in0=gt, in1=v, op=OP.mult)
            var, vco = blocked_scan(a_ap, cA, arow_ap, b2, vcar, "v")
            sq = dk.tile([128, L], f32, tag="sq")
            nc.scalar.activation(out=sq, in_=var, func=AF.Sqrt, bias=eps_t)
            al = dk.tile([128, L], bf16, tag="al")
            nc.vector.reciprocal(out=al, in_=sq)
            a_bf = rowbf.tile([1, TC], bf16, tag="arow", name="a_bf")
            nc.sync.dma_start(out=a_bf, in_=al)
            st_row_bf[k] = (m_bf, a_bf)
            return mco, vco

        def chunk_b(k):
            """normalize + affine + store for chunk k."""
            t0 = k * TC
            x_ch = st_x.pop(k)
            if mode == "noout":
                nc.scalar.dma_start(out=out_v[:, :, t0:t0 + TC], in_=x_ch)
                return
            m_bf, a_bf = st_row_bf.pop(k)
            bcm = bcp.tile([128, TC], bf16, tag="bcm", name="bcm")
            bca = bcp.tile([128, TC], bf16, tag="bca", name="bca")
            for sx in range(NSC):
                sl = slice(sx * SUB, (sx + 1) * SUB)
                pm = ps_bc.tile([128, SUB], f32, name="pm", tag="pb")
                nc.tensor.matmul(pm, ones_col, m_bf[:, sl],
                                 start=True, stop=True)
                nc.scalar.activation(out=bcm[:, sl], in_=pm, func=AF.Copy)
                pa = ps_bc.tile([128, SUB], f32, name="pa", tag="pb")
                nc.tensor.matmul(pa, ones_col, a_bf[:, sl],
                                 start=True, stop=True)
                nc.scalar.activation(out=bca[:, sl], in_=pa, func=AF.Copy)
            y = yp.tile([128, NCT, TC], bf16, name="y")
            bcm4 = bass.AP(tensor=bcm.tensor, offset=bcm.offset,
                           ap=[bcm.ap[0], [0, NCT], bcm.ap[1]])
            nc.vector.tensor_tensor(out=y, in0=x_ch, in1=bcm4,
                                    op=OP.subtract)
            z = zo.tile([128, NCT, TC], bf16, tag="zo", name="z")
            bca4 = bass.AP(tensor=bca.tensor, offset=bca.offset,
                           ap=[bca.ap[0], [0, NCT], bca.ap[1]])
            nc.vector.tensor_tensor(out=z, in0=y, in1=bca4, op=OP.mult)
            o_sb = zo.tile([128, NCT, TC], bf16, tag="zo", name="o_sb")
            for ct in range(NCT):
                nc.scalar.activation(
                    out=o_sb[:, ct, :], in_=z[:, ct, :], func=AF.Identity,
                    scale=wsc[:, ct:ct + 1], bias=wbi[:, ct:ct + 1])
            nc.scalar.dma_start(out=out_v[:, :, t0:t0 + TC], in_=o_sb)

        # Emission: a(0), d(0), a(1), b(0), d(1), a(2), b(1), d(2), a(3),
        # b(2), d(3), b(3) -- chunk_b(k) sorts before dance(k+1) so the
        # in-order DVE queue never blocks ready bulk work behind a scan
        # that is still waiting on the next chunk's reductions.
        mcar = vcar = None
        for k in range(nch):
            chunk_a(k)
            if k >= 1:
                chunk_b(k - 1)
            mcar, vcar = dance(k, mcar, vcar)
        chunk_b(nch - 1)

        if rep_cm is not None:
            rep_cm.__exit__(None, None, None)

    nc.compile()
    return nc


def _get_program(t_total=T, reps=1):
    key = (t_total, reps)
    if key not in _PROG_CACHE:
        _PROG_CACHE[key] = _build_program(t_total, reps)
    return _PROG_CACHE[key]


def _host_prep(x, g, Wa_w, Wb_w, Wo_w, Wo_b, t_total):
    """Build per-core input maps (host does only O(B*T + C) work)."""
    from concourse import mybir
    bf = mybir.dt.np(mybir.dt.bfloat16)
    x = np.asarray(x, np.float32)
    g = np.asarray(g, np.float32)
    wa = np.asarray(Wa_w, np.float32).reshape(C)
    wb = np.asarray(Wb_w, np.float32).reshape(C)
    wo_w = np.asarray(Wo_w, np.float32).reshape(C)
    wo_b = np.asarray(Wo_b, np.float32).reshape(C)

    def softmax(v):
        e = np.exp(v - v.max())
        return (e / e.sum()).astype(np.float32)

    wa, wb = softmax(wa), softmax(wb)
    zero = np.zeros_like(wa)
    w3a = np.stack([wa, wb, zero], 1).reshape(NCT, 128, 3).transpose(1, 0, 2)
    w3b = np.stack([zero, zero, wb], 1).reshape(NCT, 128, 3).transpose(1, 0, 2)
    wsc = wo_w.reshape(NCT, 128).T
    wbi = wo_b.reshape(NCT, 128).T

    shared = {
        "w3a": np.ascontiguousarray(w3a).astype(bf),
        "w3b": np.ascontiguousarray(w3b).astype(bf),
        "wscale": np.ascontiguousarray(wsc),
        "wbias": np.ascontiguousarray(wbi),
    }
    nch = t_total // TC

    def pk(v):
        return v.transpose(1, 0, 2).reshape(128, nch * L)

    in_maps = []
    for b in range(x.shape[0]):
        gt = (g[b, 0, :] * MOMENTUM).astype(np.float32)
        a = (1.0 - gt).astype(np.float32)
        gt_b = gt.reshape(nch, 128, L)
        a_b = a.reshape(nch, 128, L)
        cA = np.cumprod(a_b, axis=2).astype(np.float32)
        hrows = np.stack([pk(gt_b), pk(a_b), pk(cA)], axis=1)
        arows = np.ascontiguousarray(cA[:, :, -1].reshape(1, -1))
        in_maps.append({
            "x": np.ascontiguousarray(x[b]),
            "hrows": np.ascontiguousarray(hrows),
            "arows": arows,
            **shared,
        })
    return in_maps


LAST_RESULTS = None


def kernel(x, g, Wa_w, Wb_w, Wo_w, Wo_b):
    global LAST_RESULTS
    from concourse.bass_utils import run_bass_kernel_spmd

    t_total = x.shape[2]
    nc = _get_program(t_total)
    in_maps = _host_prep(x, g, Wa_w, Wb_w, Wo_w, Wo_b, t_total)
    n = len(in_maps)
    res = run_bass_kernel_spmd(nc, in_maps, list(range(n)))
    LAST_RESULTS = res
    return np.stack(
        [np.asarray(res.results[i]["out"]).astype(np.float32)
         for i in range(n)], 0)
